# revision 7
# baseline (speedup 1.0000x reference)
"""Trainium2 Bass kernel for a single causal self-attention head.

Reference computation (fp32):
    Q = q @ Wq; K = q @ Wk; V = q @ Wv          # q: [B, T, D]
    scores = Q K^T / sqrt(D)  (causal masked)
    out = softmax(scores) @ V                    # [B, T, dv]

Shapes hardcoded: B=512, T=200, D=1024, dk=dv=64, 8 NeuronCores,
batch-sharded 64 per core (pure data parallel, weights replicated).

Design (~67us/core vs 173us for the f32r baseline):
  - The serialized-DMA roofline dominates, so the input is shipped as
    fp8e4 (q*8, weights*32 keep operands out of the subnormal range)
    at 1792B/partition/batch; the 24-column pad of each DoubleRow
    k-block hides a bf16 copy of q rows t<12 at zero extra bytes.
  - Q,K projection: fp8 DoubleRow (contraction depth 256, 0.5
    cycles/row) over [Wq|Wk] stationaries; V likewise DoubleRow with
    qT-block stationaries directly into [t, dv] layout, then rows
    s<12 are overwritten in PSUM by a bf16 pass (causal attention
    weight concentrates on early rows at small t, where fp8 V error
    would breach the 2e-2 gate; elsewhere it averages out).
  - K^T is evacuated from PSUM rows 64:128 to SBUF rows 0:64 with a
    partition-shifted copy (engines allow mismatched base partitions;
    matmuls do not), so both E-operands sit at base partition 0.
  - E = exp(scores * 2^-21) with causal trimming (s-tile [0:128)
    streams t 0:200, s-tile [128:200) only t>=128), single merged exp
    per batch, bf16 E, masks multiplied on DVE (2x mode) / Pool.
  - U = E^T @ [V|1] accumulates numerator and softmax denominator per
    even/odd-t interleave; the device stores *unnormalized* bf16
    [vals|denom] blocks (one 520B-descriptor DMA per pair) and the
    host performs the final divide - that removes reciprocal+scaled
    copies from the device and halves output DMA bytes.
  - 4-stage software pipeline (load superpair | proj p | scores p-1 |
    U/out p-2) with engine balance per pair: Act (exp+qm evac+half the
    out-copies) ~1.58us, DVE (kt evac+V evac+e0 mask+out-copies)
    ~1.51us, Pool (SWDGE input issue+ones memsets+e1 mask) ~1.49us,
    DMA 1.45us, PE 1.49us.
"""

import numpy as np
import ml_dtypes

import concourse.bass as bass
import concourse.tile as tile
from concourse import bacc, mybir
from concourse.bass_utils import run_bass_kernel_spmd

B, T, D = 512, 200, 1024
DK = 64
N_CORES = 8
B_CORE = B // N_CORES  # 64
ND = D // 128  # 8 d-tiles
F32 = mybir.dt.float32
F32R = mybir.dt.float32r
BF16 = mybir.dt.bfloat16
FP8 = mybir.dt.float8e4
U8 = mybir.dt.uint8

T0W = 128  # first t/s tile width
T1W = T - T0W  # 72
TH = 12  # bf16 "head" rows (V for s < TH is computed exactly)
T1P = 96  # second t-tile padded from 72 to 96 (DoubleRow M must be 32k)
TP = T0W + T1P  # 224 padded t extent
KB = 2 * TP  # 448 bytes per k-tile block
XB = (ND // 2) * KB  # 1792 bytes per partition; the bf16 head hides in
# the 24-column tt1 pad of each (k, j) slot (24B = 12 bf16 each)
ESCALE = 1.0 / (32.0 * 65536.0)  # scores arrive x(8*32)^2


def build_nc(n_batch=B_CORE, repeat=1):
    """Build the per-core Bass module. Same program on all cores (SPMD)."""
    nc = bacc.Bacc("TRN2")

    # qT host-prepped as [b, 128, ND*T] bf16: partition p holds d-rows
    # d*128+p of q^T, contiguous per partition (3200B descriptors).
    qx = nc.dram_tensor("qx", [n_batch, 128, XB], U8, kind="ExternalInput")
    wqk = nc.dram_tensor("wqk", [D, 128], FP8, kind="ExternalInput")
    wv8 = nc.dram_tensor("wv8", [D, DK], FP8, kind="ExternalInput")
    wv16 = nc.dram_tensor("wv16", [D, DK], BF16, kind="ExternalInput")
    mask0 = nc.dram_tensor("mask0", [128, T0W], BF16, kind="ExternalInput")
    mask1 = nc.dram_tensor("mask1", [T1W, T1W], BF16, kind="ExternalInput")
    out = nc.dram_tensor("out", [n_batch // 2, 100, 260], BF16, kind="ExternalOutput")

    assert n_batch % 2 == 0
    n_pair = n_batch // 2

    with tile.TileContext(nc) as tc:
        with (
            tc.tile_pool(name="singles", bufs=1) as singles,
            tc.tile_pool(name="qt", bufs=4) as qt_pool,
            tc.tile_pool(name="qk", bufs=3) as qk_pool,
            tc.tile_pool(name="esb", bufs=8) as esb_pool,
            tc.tile_pool(name="vsb", bufs=6) as vsb_pool,
            tc.tile_pool(name="osb", bufs=8) as osb_pool,
            tc.tile_pool(name="ps_qk", bufs=2, space="PSUM") as ps_qk,
            tc.tile_pool(name="ps_v", bufs=2, space="PSUM") as ps_v,
            tc.tile_pool(name="ps_e", bufs=2, space="PSUM") as ps_e,
            tc.tile_pool(name="ps_u", bufs=2, space="PSUM") as ps_u,
        ):
            # ---- constants, loaded once ----
            wqk_sb = singles.tile([128, ND, 128], FP8)
            nc.sync.dma_start(
                out=wqk_sb, in_=wqk.rearrange("(d p) j -> p d j", p=128)
            )
            wv8_sb = singles.tile([128, ND, DK], FP8)
            nc.sync.dma_start(out=wv8_sb, in_=wv8.rearrange("(d p) j -> p d j", p=128))
            wv16_sb = singles.tile([128, ND, DK], BF16)
            nc.sync.dma_start(
                out=wv16_sb, in_=wv16.rearrange("(d p) j -> p d j", p=128)
            )
            m0_sb = singles.tile([128, T0W], BF16)
            nc.sync.dma_start(out=m0_sb, in_=mask0[:, :])
            m1_sb = singles.tile([T1W, T1W], BF16)
            nc.sync.dma_start(out=m1_sb, in_=mask1[:, :])

            # persistent V-moving buffers: ones columns written once
            va_bufs = []
            for i in range(6):
                vb = singles.tile([128, 130], BF16, tag=f"vab{i}", name=f"vab{i}")
                va_bufs.append(vb)
                nc.gpsimd.memset(vb[:, 64:65], 1.0)
                nc.gpsimd.memset(vb[:, 129:130], 1.0)

            def emit_load(b0, nb2):
                """Prefetch batches [b0, b0+nb2) via the Pool SWDGE queue."""
                qt = qt_pool.tile([128, nb2, XB], U8, tag=f"qt{nb2}")
                nc.gpsimd.dma_start(
                    out=qt, in_=qx[b0 : b0 + nb2].rearrange("b p x -> p b x")
                )
                return qt

            def emit_projection(p, qt):
                """Project Q,K and V (fp8 DoubleRow; V rows t<TH redone in
                bf16), evacuate. Returns attention inputs."""
                # fp8 per-partition layout: 4 k-tiles x 400B, each k-tile
                # [j*128+t | 256 + j*72 + (t-128)] so DoubleRow stationaries
                # are contiguous [ [tw,2], [1,tw] ] blocks (ISA requirement).
                q8a = qt.bitcast(FP8).rearrange(
                    "pp b (k y) -> pp k b y", k=ND // 2
                )
                qk_ps = ps_qk.tile([128, 2 * TP], F32)
                qk3 = qk_ps.rearrange("m (b t) -> m b t", b=2)
                for ti, (t0, tw) in enumerate(((0, T0W), (T0W, T1P))):
                    y0 = 0 if ti == 0 else 2 * T0W
                    for k in range(ND // 2):
                        nc.tensor.matmul(
                            qk3[:, :, t0 : t0 + tw],
                            wqk_sb[:, 2 * k : 2 * k + 2, :],
                            q8a[:, k, :, y0 : y0 + 2 * tw].rearrange(
                                "pp b (j t) -> pp j b t", j=2
                            ),
                            start=(k == 0), stop=(k == ND // 2 - 1),
                            perf_mode=mybir.MatmulPerfMode.DoubleRow,
                            skip_group_check=True,
                        )

                # V in [t, dv] rows: fp8 DoubleRow, then bf16 head overwrite
                # (col 64 / 129 are gap columns the ones-memsets fill)
                v_ps = ps_v.tile([128, 2, 130], F32)
                for bi in range(2):
                    for ti, (t0, tw) in enumerate(((0, T0W), (T0W, T1P))):
                        dst = v_ps[0:tw, bi, 65 * ti : 65 * ti + 64]
                        y0 = 0 if ti == 0 else 2 * T0W
                        for k in range(ND // 2):
                            nc.tensor.matmul(
                                dst,
                                q8a[:, k, bi, y0 : y0 + 2 * tw].rearrange(
                                    "pp (j t) -> pp j t", j=2
                                ),
                                wv8_sb[:, 2 * k : 2 * k + 2, :],
                                start=(k == 0), stop=(k == ND // 2 - 1),
                                perf_mode=mybir.MatmulPerfMode.DoubleRow,
                            )
                    for d in range(ND):
                        k, j = d // 2, d % 2
                        h0 = k * KB + 2 * T0W + j * T1P + T1W
                        nc.tensor.matmul(
                            v_ps[0:TH, bi, 0:64],
                            qt[:, bi, h0 : h0 + 2 * TH].bitcast(BF16),
                            wv16_sb[:, d, :],
                            start=(d == 0), stop=(d == ND - 1),
                            skip_group_check=True,
                        )

                # evacuate: Q^T as bf16 moving operand, K^T partition-shifted
                # down to rows 0:64 as the stationary operand
                qk4 = qk_ps.rearrange("m (b t) -> m b t", b=2)
                qm_sb = qk_pool.tile([64, 2, T], BF16, tag="qm")
                nc.scalar.copy(qm_sb, qk4[0:64, :, 0:T])
                kt_sb = qk_pool.tile([64, 2, T], BF16, tag="kt")
                nc.vector.tensor_copy(kt_sb, qk4[64:128, :, 0:T])

                # 256*V + ones columns -> moving operand [v0 | 1 | v1 | 1]
                # (x256 scale divided out on the host; ones are persistent)
                vaugs = []
                for bi in range(2):
                    va = va_bufs[(2 * p + bi) % 6]
                    nc.vector.tensor_copy(va[:, 0:64], v_ps[:, bi, 0:64])
                    nc.vector.tensor_copy(
                        va[0:T1W, 65:129], v_ps[0:T1W, bi, 65:129]
                    )
                    vaugs.append(va)
                return qm_sb, kt_sb, vaugs

            def emit_scores(p, qm_sb, kt_sb):
                """E = exp(K Q^T / 32), causally trimmed + masked, bf16."""
                es = []
                for bi in range(2):
                    e_ps = ps_e.tile([128, T + T1W], F32, tag="e_ps")
                    nc.tensor.matmul(
                        e_ps[:, 0:T],
                        kt_sb[:, bi, 0:T0W],
                        qm_sb[:, bi, 0:T],
                        start=True, stop=True,
                    )
                    nc.tensor.matmul(
                        e_ps[0:T1W, T : T + T1W],
                        kt_sb[:, bi, T0W:T],
                        qm_sb[:, bi, T0W:T],
                        start=True, stop=True,
                    )
                    eb = esb_pool.tile([128, T + T1W], BF16, tag="eb")
                    nc.scalar.activation(
                        eb, e_ps,
                        mybir.ActivationFunctionType.Exp, scale=ESCALE,
                    )
                    e0 = eb[:, 0:T]
                    e1 = eb[0:T1W, T : T + T1W]
                    nc.vector.tensor_mul(e0[:, 0:T0W], e0[:, 0:T0W], m0_sb)
                    nc.gpsimd.tensor_mul(e1, e1, m1_sb)
                    es.append((e0, e1))
                return es

            def emit_out(p, es, vaugs):
                """U = E^T @ [V|1]; store unnormalized bf16 (host divides)."""
                o_sb = osb_pool.tile([100, 260], BF16, tag="o_sb")
                u_ps = ps_u.tile([100, 260], F32, tag="u_ps")
                for bi in range(2):
                    e0, e1 = es[bi]
                    e0p = e0.rearrange("s (t c) -> s t c", c=2)
                    e1p = e1.rearrange("s (t c) -> s t c", c=2)
                    va = vaugs[bi]
                    u2 = u_ps[:, 130 * bi : 130 * bi + 130]
                    for ci in range(2):
                        nc.tensor.matmul(
                            u2[:, 65 * ci : 65 * ci + 65],
                            e0p[:, :, ci],
                            va[:, 0:65],
                            start=True, stop=False,
                            skip_group_check=True,
                        )
                        nc.tensor.matmul(
                            u2[64:100, 65 * ci : 65 * ci + 65],
                            e1p[:, :, ci],
                            va[0:T1W, 65:130],
                            start=False, stop=True,
                            skip_group_check=True,
                        )
                if p % 2 == 0:
                    nc.scalar.copy(o_sb, u_ps)
                else:
                    nc.vector.tensor_copy(o_sb, u_ps)
                nc.sync.dma_start(
                    out=out[p].rearrange("t (b x) -> t b x", b=2), in_=o_sb
                )

            # software pipeline, 4 stages 1 pair apart:
            # load superpair | proj p | scores p-1 | U/out p-2
            for _rep in range(repeat):
                proj_t = {}
                score_t = {}
                q_tiles = {0: emit_load(0, 2)}
                if n_pair > 1:
                    t4 = emit_load(2, 4)
                    q_tiles[1] = t4[:, 0:2, :]
                    q_tiles[2] = t4[:, 2:4, :]
                for p in range(n_pair + 2):
                    if p < n_pair:
                        if p >= 1 and p % 2 == 1 and p + 2 < n_pair:
                            if p + 3 < n_pair:
                                t4 = emit_load(2 * (p + 2), 4)
                                q_tiles[p + 2] = t4[:, 0:2, :]
                                q_tiles[p + 3] = t4[:, 2:4, :]
                            else:
                                q_tiles[p + 2] = emit_load(2 * (p + 2), 2)
                        proj_t[p] = emit_projection(p, q_tiles.pop(p))
                    if p - 2 >= 0:
                        emit_out(p - 2, score_t.pop(p - 2),
                                 proj_t.pop(p - 2)[2])
                    if p - 1 >= 0 and p - 1 < n_pair:
                        qm_sb, kt_sb, _ = proj_t[p - 1]
                        score_t[p - 1] = emit_scores(p - 1, qm_sb, kt_sb)

    nc.compile()
    return nc


def round_f32r(a):
    """Round fp32 to the PE's fp32r format (11-bit mantissa, RNE)."""
    b = np.ascontiguousarray(a, dtype=np.float32).view(np.uint32)
    r = (b + 0x7FF + ((b >> 12) & 1)) & np.uint32(0xFFFFF000)
    return r.astype(np.uint32).view(np.float32)


def _host_inputs(q, Wq, Wk, Wv):
    """Shared (replicated) device inputs + per-core packed qx shards."""
    wqk = np.ascontiguousarray(
        np.concatenate([Wq, Wk], axis=1) * 32.0, dtype=np.float32
    ).astype(ml_dtypes.float8_e4m3fn)
    wv32 = np.ascontiguousarray(np.asarray(Wv) * 32.0, dtype=np.float32)
    wv8 = wv32.astype(ml_dtypes.float8_e4m3fn)
    wv16 = wv32.astype(ml_dtypes.bfloat16)
    t_idx = np.arange(T0W)[None, :]
    m0 = (t_idx >= np.arange(128)[:, None]).astype(ml_dtypes.bfloat16)
    t_idx = (T0W + np.arange(T1W))[None, :]
    m1 = (t_idx >= (T0W + np.arange(T1W))[:, None]).astype(ml_dtypes.bfloat16)
    # qT[b, p, d8, t]: partition p holds rows d8*128+p of q^T
    nb = q.shape[0]
    qT = (
        np.asarray(q, dtype=np.float32)
        .transpose(0, 2, 1)
        .reshape(nb, ND, 128, T)
        .transpose(0, 2, 1, 3)
    )  # [b, 128, ND, T]
    q8 = (qT * 8.0).astype(ml_dtypes.float8_e4m3fn)  # [b, 128, 8, 200]
    qa = q8[..., 0:T0W].reshape(nb, 128, ND // 2, 2 * T0W)
    # tt1 block per (k, j): [72 fp8 | 24B pad = 12 bf16 head values]
    qb = np.zeros((nb, 128, ND, T1P), dtype=np.uint8)
    qb[..., 0:T1W] = q8[..., T0W:T].view(np.uint8)
    q16 = np.ascontiguousarray((qT[:, :, :, 0:TH] * 8.0).astype(ml_dtypes.bfloat16))
    qb[..., T1W:T1P] = q16.view(np.uint8).reshape(nb, 128, ND, 2 * TH)
    qb = qb.reshape(nb, 128, ND // 2, 2 * T1P)
    qx = np.concatenate([qa.view(np.uint8), qb], axis=3).reshape(nb, 128, XB)
    return np.ascontiguousarray(qx), {
        "wqk": wqk, "wv8": wv8, "wv16": wv16, "mask0": m0, "mask1": m1,
    }


_NC_CACHE = {}


def _get_nc(n_batch=B_CORE, repeat=1):
    key = (n_batch, repeat)
    if key not in _NC_CACHE:
        _NC_CACHE[key] = build_nc(n_batch, repeat)
    return _NC_CACHE[key]


def _unpack_out(u):
    """[n_pair, 100, 260] bf16 unnormalized -> [2*n_pair, T, DK] f32."""
    u = u.astype(np.float32).reshape(u.shape[0], 100, 2, 2, 65)
    vals = u[..., 0:64]  # [p, t2, b, ci, 64] = 256 * numerator
    den = u[..., 64:65]
    o = vals / den / 256.0  # [p, t2, b, ci, 64]
    o = o.transpose(0, 2, 1, 3, 4).reshape(u.shape[0] * 2, T, DK)
    return np.ascontiguousarray(o)


def kernel(q, Wq, Wk, Wv):
    q = np.asarray(q, dtype=np.float32)
    qT, shared = _host_inputs(q, np.asarray(Wq), np.asarray(Wk), np.asarray(Wv))

    nc = _get_nc()
    in_maps = [
        {"qx": np.ascontiguousarray(qT[c * B_CORE : (c + 1) * B_CORE]), **shared}
        for c in range(N_CORES)
    ]
    res = run_bass_kernel_spmd(nc, in_maps, core_ids=list(range(N_CORES)))
    return np.concatenate([_unpack_out(r["out"]) for r in res.results], axis=0)


# revision 8
# speedup vs baseline: 1.0135x; 1.0135x over previous
"""Trainium2 Bass kernel for a single causal self-attention head.

Reference computation (fp32):
    Q = q @ Wq; K = q @ Wk; V = q @ Wv          # q: [B, T, D]
    scores = Q K^T / sqrt(D)  (causal masked)
    out = softmax(scores) @ V                    # [B, T, dv]

Shapes hardcoded: B=512, T=200, D=1024, dk=dv=64, 8 NeuronCores,
batch-sharded 64 per core (pure data parallel, weights replicated).

Design (~67us/core vs 173us for the f32r baseline):
  - The serialized-DMA roofline dominates, so the input is shipped as
    fp8e4 (q*8, weights*32 keep operands out of the subnormal range)
    at 1792B/partition/batch; the 24-column pad of each DoubleRow
    k-block hides a bf16 copy of q rows t<12 at zero extra bytes.
  - Q,K projection: fp8 DoubleRow (contraction depth 256, 0.5
    cycles/row) over [Wq|Wk] stationaries; V likewise DoubleRow with
    qT-block stationaries directly into [t, dv] layout, then rows
    s<12 are overwritten in PSUM by a bf16 pass (causal attention
    weight concentrates on early rows at small t, where fp8 V error
    would breach the 2e-2 gate; elsewhere it averages out).
  - K^T is evacuated from PSUM rows 64:128 to SBUF rows 0:64 with a
    partition-shifted copy (engines allow mismatched base partitions;
    matmuls do not), so both E-operands sit at base partition 0.
  - E = exp(scores * 2^-21) with causal trimming (s-tile [0:128)
    streams t 0:200, s-tile [128:200) only t>=128), single merged exp
    per batch, bf16 E, masks multiplied on DVE (2x mode) / Pool.
  - U = E^T @ [V|1] accumulates numerator and softmax denominator per
    even/odd-t interleave; the device stores *unnormalized* bf16
    [vals|denom] blocks (one 520B-descriptor DMA per pair) and the
    host performs the final divide - that removes reciprocal+scaled
    copies from the device and halves output DMA bytes.
  - 4-stage software pipeline (load superpair | proj p | scores p-1 |
    U/out p-2) with engine balance per pair: Act (exp+qm evac+half the
    out-copies) ~1.58us, DVE (kt evac+V evac+e0 mask+out-copies)
    ~1.51us, Pool (SWDGE input issue+ones memsets+e1 mask) ~1.49us,
    DMA 1.45us, PE 1.49us.
"""

import numpy as np
import ml_dtypes

import concourse.bass as bass
import concourse.tile as tile
from concourse import bacc, mybir
from concourse.bass_utils import run_bass_kernel_spmd

B, T, D = 512, 200, 1024
DK = 64
N_CORES = 8
B_CORE = B // N_CORES  # 64
ND = D // 128  # 8 d-tiles
F32 = mybir.dt.float32
F32R = mybir.dt.float32r
BF16 = mybir.dt.bfloat16
FP8 = mybir.dt.float8e4
U8 = mybir.dt.uint8

T0W = 128  # first t/s tile width
T1W = T - T0W  # 72
TH = 12  # bf16 "head" rows (V for s < TH is computed exactly)
T1P = 96  # second t-tile padded from 72 to 96 (DoubleRow M must be 32k)
TP = T0W + T1P  # 224 padded t extent
KB = 2 * TP  # 448 bytes per k-tile block
XB = (ND // 2) * KB  # 1792 bytes per partition; the bf16 head hides in
# the 24-column tt1 pad of each (k, j) slot (24B = 12 bf16 each)
ESCALE = 1.0 / (32.0 * 65536.0)  # scores arrive x(8*32)^2


def build_nc(n_batch=B_CORE, repeat=1):
    """Build the per-core Bass module. Same program on all cores (SPMD)."""
    nc = bacc.Bacc("TRN2")

    # qT host-prepped as [b, 128, ND*T] bf16: partition p holds d-rows
    # d*128+p of q^T, contiguous per partition (3200B descriptors).
    qx = nc.dram_tensor("qx", [n_batch, 128, XB], U8, kind="ExternalInput")
    wqk = nc.dram_tensor("wqk", [D, 128], FP8, kind="ExternalInput")
    wv8 = nc.dram_tensor("wv8", [D, DK], FP8, kind="ExternalInput")
    wv16 = nc.dram_tensor("wv16", [D, DK], BF16, kind="ExternalInput")
    mask0 = nc.dram_tensor("mask0", [128, T0W], BF16, kind="ExternalInput")
    mask1 = nc.dram_tensor("mask1", [T1W, T1W], BF16, kind="ExternalInput")
    out = nc.dram_tensor("out", [n_batch // 2, 100, 260], BF16, kind="ExternalOutput")

    assert n_batch % 2 == 0
    n_pair = n_batch // 2

    with tile.TileContext(nc) as tc:
        with (
            tc.tile_pool(name="singles", bufs=1) as singles,
            tc.tile_pool(name="qt", bufs=4) as qt_pool,
            tc.tile_pool(name="qk", bufs=3) as qk_pool,
            tc.tile_pool(name="esb", bufs=8) as esb_pool,
            tc.tile_pool(name="vsb", bufs=6) as vsb_pool,
            tc.tile_pool(name="osb", bufs=8) as osb_pool,
            tc.tile_pool(name="ps_qk", bufs=2, space="PSUM") as ps_qk,
            tc.tile_pool(name="ps_v", bufs=2, space="PSUM") as ps_v,
            tc.tile_pool(name="ps_e", bufs=2, space="PSUM") as ps_e,
            tc.tile_pool(name="ps_u", bufs=2, space="PSUM") as ps_u,
        ):
            # ---- constants, loaded once ----
            wqk_sb = singles.tile([128, ND, 128], FP8)
            nc.sync.dma_start(
                out=wqk_sb, in_=wqk.rearrange("(d p) j -> p d j", p=128)
            )
            wv8_sb = singles.tile([128, ND, DK], FP8)
            nc.sync.dma_start(out=wv8_sb, in_=wv8.rearrange("(d p) j -> p d j", p=128))
            wv16_sb = singles.tile([128, ND, DK], BF16)
            nc.sync.dma_start(
                out=wv16_sb, in_=wv16.rearrange("(d p) j -> p d j", p=128)
            )
            m0_sb = singles.tile([128, T0W], BF16)
            nc.sync.dma_start(out=m0_sb, in_=mask0[:, :])
            m1_sb = singles.tile([T1W, T1W], BF16)
            nc.sync.dma_start(out=m1_sb, in_=mask1[:, :])

            def emit_load(b0, nb2):
                """Prefetch batches [b0, b0+nb2) via the Pool SWDGE queue."""
                qt = qt_pool.tile([128, nb2, XB], U8, tag=f"qt{nb2}")
                nc.gpsimd.dma_start(
                    out=qt, in_=qx[b0 : b0 + nb2].rearrange("b p x -> p b x")
                )
                return qt

            def emit_projection(p, qt):
                """Project Q,K and V (fp8 DoubleRow; V rows t<TH redone in
                bf16), evacuate. Returns attention inputs."""
                # fp8 per-partition layout: 4 k-tiles x 400B, each k-tile
                # [j*128+t | 256 + j*72 + (t-128)] so DoubleRow stationaries
                # are contiguous [ [tw,2], [1,tw] ] blocks (ISA requirement).
                q8a = qt.bitcast(FP8).rearrange(
                    "pp b (k y) -> pp k b y", k=ND // 2
                )
                qk_ps = ps_qk.tile([128, 2 * TP], F32)
                qk3 = qk_ps.rearrange("m (b t) -> m b t", b=2)
                for ti, (t0, tw) in enumerate(((0, T0W), (T0W, T1P))):
                    y0 = 0 if ti == 0 else 2 * T0W
                    for k in range(ND // 2):
                        nc.tensor.matmul(
                            qk3[:, :, t0 : t0 + tw],
                            wqk_sb[:, 2 * k : 2 * k + 2, :],
                            q8a[:, k, :, y0 : y0 + 2 * tw].rearrange(
                                "pp b (j t) -> pp j b t", j=2
                            ),
                            start=(k == 0), stop=(k == ND // 2 - 1),
                            perf_mode=mybir.MatmulPerfMode.DoubleRow,
                            skip_group_check=True,
                        )

                # V in [t, dv] rows: fp8 DoubleRow, then bf16 head overwrite
                # (col 64 / 129 are gap columns the ones-memsets fill)
                v_ps = ps_v.tile([128, 2, 130], F32)
                for bi in range(2):
                    for ti, (t0, tw) in enumerate(((0, T0W), (T0W, T1P))):
                        dst = v_ps[0:tw, bi, 65 * ti : 65 * ti + 64]
                        y0 = 0 if ti == 0 else 2 * T0W
                        for k in range(ND // 2):
                            nc.tensor.matmul(
                                dst,
                                q8a[:, k, bi, y0 : y0 + 2 * tw].rearrange(
                                    "pp (j t) -> pp j t", j=2
                                ),
                                wv8_sb[:, 2 * k : 2 * k + 2, :],
                                start=(k == 0), stop=(k == ND // 2 - 1),
                                perf_mode=mybir.MatmulPerfMode.DoubleRow,
                            )
                    for d in range(ND):
                        k, j = d // 2, d % 2
                        h0 = k * KB + 2 * T0W + j * T1P + T1W
                        nc.tensor.matmul(
                            v_ps[0:TH, bi, 0:64],
                            qt[:, bi, h0 : h0 + 2 * TH].bitcast(BF16),
                            wv16_sb[:, d, :],
                            start=(d == 0), stop=(d == ND - 1),
                            skip_group_check=True,
                        )

                # evacuate: Q^T as bf16 moving operand, K^T partition-shifted
                # down to rows 0:64 as the stationary operand
                qk4 = qk_ps.rearrange("m (b t) -> m b t", b=2)
                qm_sb = qk_pool.tile([64, 2, T], BF16, tag="qm")
                nc.scalar.copy(qm_sb, qk4[0:64, :, 0:T])
                kt_sb = qk_pool.tile([64, 2, T], BF16, tag="kt")
                nc.vector.tensor_copy(kt_sb, qk4[64:128, :, 0:T])

                # 256*V + ones columns -> moving operand [v0 | 1 | v1 | 1]
                # (the x256 scale is divided out on the host)
                vaugs = []
                for bi in range(2):
                    va = vsb_pool.tile([128, 130], BF16, tag=f"va{bi}")
                    nc.vector.tensor_copy(va, v_ps[:, bi, :])
                    nc.gpsimd.memset(va[:, 64:65], 1.0)
                    nc.gpsimd.memset(va[0:T1W, 129:130], 1.0)
                    vaugs.append(va)
                return qm_sb, kt_sb, vaugs

            def emit_scores(p, qm_sb, kt_sb):
                """E = exp(K Q^T / 32), causally trimmed + masked, bf16."""
                es = []
                for bi in range(2):
                    e_ps = ps_e.tile([128, T + T1W], F32, tag="e_ps")
                    nc.tensor.matmul(
                        e_ps[:, 0:T],
                        kt_sb[:, bi, 0:T0W],
                        qm_sb[:, bi, 0:T],
                        start=True, stop=True,
                    )
                    nc.tensor.matmul(
                        e_ps[0:T1W, T : T + T1W],
                        kt_sb[:, bi, T0W:T],
                        qm_sb[:, bi, T0W:T],
                        start=True, stop=True,
                    )
                    eb = esb_pool.tile([128, T + T1W], BF16, tag="eb")
                    nc.scalar.activation(
                        eb, e_ps,
                        mybir.ActivationFunctionType.Exp, scale=ESCALE,
                    )
                    e0 = eb[:, 0:T]
                    e1 = eb[0:T1W, T : T + T1W]
                    nc.vector.tensor_mul(e0[:, 0:T0W], e0[:, 0:T0W], m0_sb)
                    nc.gpsimd.tensor_mul(e1, e1, m1_sb)
                    es.append((e0, e1))
                return es

            def emit_out(p, es, vaugs):
                """U = E^T @ [V|1]; store unnormalized bf16 (host divides)."""
                o_sb = osb_pool.tile([100, 260], BF16, tag="o_sb")
                u_ps = ps_u.tile([100, 260], F32, tag="u_ps")
                for bi in range(2):
                    e0, e1 = es[bi]
                    e0p = e0.rearrange("s (t c) -> s t c", c=2)
                    e1p = e1.rearrange("s (t c) -> s t c", c=2)
                    va = vaugs[bi]
                    u2 = u_ps[:, 130 * bi : 130 * bi + 130]
                    for ci in range(2):
                        nc.tensor.matmul(
                            u2[:, 65 * ci : 65 * ci + 65],
                            e0p[:, :, ci],
                            va[:, 0:65],
                            start=True, stop=False,
                            skip_group_check=True,
                        )
                        nc.tensor.matmul(
                            u2[64:100, 65 * ci : 65 * ci + 65],
                            e1p[:, :, ci],
                            va[0:T1W, 65:130],
                            start=False, stop=True,
                            skip_group_check=True,
                        )
                if p % 2 == 0:
                    nc.scalar.copy(o_sb, u_ps)
                else:
                    nc.vector.tensor_copy(o_sb, u_ps)
                nc.sync.dma_start(
                    out=out[p].rearrange("t (b x) -> t b x", b=2), in_=o_sb
                )

            # software pipeline, 4 stages 1 pair apart:
            # load superpair | proj p | scores p-1 | U/out p-2
            for _rep in range(repeat):
                proj_t = {}
                score_t = {}
                q_tiles = {0: emit_load(0, 2)}
                if n_pair > 1:
                    t4 = emit_load(2, 4)
                    q_tiles[1] = t4[:, 0:2, :]
                    q_tiles[2] = t4[:, 2:4, :]
                for p in range(n_pair + 2):
                    if p < n_pair:
                        if p >= 1 and (p - 1) % 4 == 0 and p + 2 < n_pair:
                            nb2 = min(8, 2 * (n_pair - p - 2))
                            t8 = emit_load(2 * (p + 2), nb2)
                            for k in range(nb2 // 2):
                                q_tiles[p + 2 + k] = t8[:, 2 * k : 2 * k + 2, :]
                        proj_t[p] = emit_projection(p, q_tiles.pop(p))
                    if p - 2 >= 0:
                        emit_out(p - 2, score_t.pop(p - 2),
                                 proj_t.pop(p - 2)[2])
                    if p - 1 >= 0 and p - 1 < n_pair:
                        qm_sb, kt_sb, _ = proj_t[p - 1]
                        score_t[p - 1] = emit_scores(p - 1, qm_sb, kt_sb)

    nc.compile()
    return nc


def round_f32r(a):
    """Round fp32 to the PE's fp32r format (11-bit mantissa, RNE)."""
    b = np.ascontiguousarray(a, dtype=np.float32).view(np.uint32)
    r = (b + 0x7FF + ((b >> 12) & 1)) & np.uint32(0xFFFFF000)
    return r.astype(np.uint32).view(np.float32)


def _host_inputs(q, Wq, Wk, Wv):
    """Shared (replicated) device inputs + per-core packed qx shards."""
    wqk = np.ascontiguousarray(
        np.concatenate([Wq, Wk], axis=1) * 32.0, dtype=np.float32
    ).astype(ml_dtypes.float8_e4m3fn)
    wv32 = np.ascontiguousarray(np.asarray(Wv) * 32.0, dtype=np.float32)
    wv8 = wv32.astype(ml_dtypes.float8_e4m3fn)
    wv16 = wv32.astype(ml_dtypes.bfloat16)
    t_idx = np.arange(T0W)[None, :]
    m0 = (t_idx >= np.arange(128)[:, None]).astype(ml_dtypes.bfloat16)
    t_idx = (T0W + np.arange(T1W))[None, :]
    m1 = (t_idx >= (T0W + np.arange(T1W))[:, None]).astype(ml_dtypes.bfloat16)
    # qT[b, p, d8, t]: partition p holds rows d8*128+p of q^T
    nb = q.shape[0]
    qT = (
        np.asarray(q, dtype=np.float32)
        .transpose(0, 2, 1)
        .reshape(nb, ND, 128, T)
        .transpose(0, 2, 1, 3)
    )  # [b, 128, ND, T]
    q8 = (qT * 8.0).astype(ml_dtypes.float8_e4m3fn)  # [b, 128, 8, 200]
    qa = q8[..., 0:T0W].reshape(nb, 128, ND // 2, 2 * T0W)
    # tt1 block per (k, j): [72 fp8 | 24B pad = 12 bf16 head values]
    qb = np.zeros((nb, 128, ND, T1P), dtype=np.uint8)
    qb[..., 0:T1W] = q8[..., T0W:T].view(np.uint8)
    q16 = np.ascontiguousarray((qT[:, :, :, 0:TH] * 8.0).astype(ml_dtypes.bfloat16))
    qb[..., T1W:T1P] = q16.view(np.uint8).reshape(nb, 128, ND, 2 * TH)
    qb = qb.reshape(nb, 128, ND // 2, 2 * T1P)
    qx = np.concatenate([qa.view(np.uint8), qb], axis=3).reshape(nb, 128, XB)
    return np.ascontiguousarray(qx), {
        "wqk": wqk, "wv8": wv8, "wv16": wv16, "mask0": m0, "mask1": m1,
    }


_NC_CACHE = {}


def _get_nc(n_batch=B_CORE, repeat=1):
    key = (n_batch, repeat)
    if key not in _NC_CACHE:
        _NC_CACHE[key] = build_nc(n_batch, repeat)
    return _NC_CACHE[key]


def _unpack_out(u):
    """[n_pair, 100, 260] bf16 unnormalized -> [2*n_pair, T, DK] f32."""
    u = u.astype(np.float32).reshape(u.shape[0], 100, 2, 2, 65)
    vals = u[..., 0:64]  # [p, t2, b, ci, 64] = 256 * numerator
    den = u[..., 64:65]
    o = vals / den / 256.0  # [p, t2, b, ci, 64]
    o = o.transpose(0, 2, 1, 3, 4).reshape(u.shape[0] * 2, T, DK)
    return np.ascontiguousarray(o)


def kernel(q, Wq, Wk, Wv):
    q = np.asarray(q, dtype=np.float32)
    qT, shared = _host_inputs(q, np.asarray(Wq), np.asarray(Wk), np.asarray(Wv))

    nc = _get_nc()
    in_maps = [
        {"qx": np.ascontiguousarray(qT[c * B_CORE : (c + 1) * B_CORE]), **shared}
        for c in range(N_CORES)
    ]
    res = run_bass_kernel_spmd(nc, in_maps, core_ids=list(range(N_CORES)))
    return np.concatenate([_unpack_out(r["out"]) for r in res.results], axis=0)


# revision 9
# speedup vs baseline: 1.0870x; 1.0725x over previous
"""Trainium2 Bass kernel for a single causal self-attention head.

Reference computation (fp32):
    Q = q @ Wq; K = q @ Wk; V = q @ Wv          # q: [B, T, D]
    scores = Q K^T / sqrt(D)  (causal masked)
    out = softmax(scores) @ V                    # [B, T, dv]

Shapes hardcoded: B=512, T=200, D=1024, dk=dv=64, 8 NeuronCores,
batch-sharded 64 per core (pure data parallel, weights replicated).

Design (~67us/core vs 173us for the f32r baseline):
  - The serialized-DMA roofline dominates, so the input is shipped as
    fp8e4 (q*8, weights*32 keep operands out of the subnormal range)
    at 1792B/partition/batch; the 24-column pad of each DoubleRow
    k-block hides a bf16 copy of q rows t<12 at zero extra bytes.
  - Q,K projection: fp8 DoubleRow (contraction depth 256, 0.5
    cycles/row) over [Wq|Wk] stationaries; V likewise DoubleRow with
    qT-block stationaries directly into [t, dv] layout, then rows
    s<12 are overwritten in PSUM by a bf16 pass (causal attention
    weight concentrates on early rows at small t, where fp8 V error
    would breach the 2e-2 gate; elsewhere it averages out).
  - K^T is evacuated from PSUM rows 64:128 to SBUF rows 0:64 with a
    partition-shifted copy (engines allow mismatched base partitions;
    matmuls do not), so both E-operands sit at base partition 0.
  - E = exp(scores * 2^-21) with causal trimming (s-tile [0:128)
    streams t 0:200, s-tile [128:200) only t>=128), single merged exp
    per batch, bf16 E, masks multiplied on DVE (2x mode) / Pool.
  - U = E^T @ [V|1] accumulates numerator and softmax denominator per
    even/odd-t interleave; the device stores *unnormalized* bf16
    [vals|denom] blocks (one 520B-descriptor DMA per pair) and the
    host performs the final divide - that removes reciprocal+scaled
    copies from the device and halves output DMA bytes.
  - 4-stage software pipeline (load superpair | proj p | scores p-1 |
    U/out p-2) with engine balance per pair: Act (exp+qm evac+half the
    out-copies) ~1.58us, DVE (kt evac+V evac+e0 mask+out-copies)
    ~1.51us, Pool (SWDGE input issue+ones memsets+e1 mask) ~1.49us,
    DMA 1.45us, PE 1.49us.
"""

import numpy as np
import ml_dtypes

import concourse.bass as bass
import concourse.tile as tile
from concourse import bacc, mybir
from concourse.bass_utils import run_bass_kernel_spmd

B, T, D = 512, 200, 1024
DK = 64
N_CORES = 8
B_CORE = B // N_CORES  # 64
ND = D // 128  # 8 d-tiles
F32 = mybir.dt.float32
F32R = mybir.dt.float32r
BF16 = mybir.dt.bfloat16
FP8 = mybir.dt.float8e4
U8 = mybir.dt.uint8

T0W = 128  # first t/s tile width
T1W = T - T0W  # 72
TH = 12  # bf16 "head" rows (V for s < TH is computed exactly)
T1P = 96  # second t-tile padded from 72 to 96 (DoubleRow M must be 32k)
TP = T0W + T1P  # 224 padded t extent
KB = 2 * TP  # 448 bytes per k-tile block
XB = (ND // 2) * KB  # 1792 bytes per partition; the bf16 head hides in
# the 24-column tt1 pad of each (k, j) slot (24B = 12 bf16 each)
ESCALE = 1.0 / (32.0 * 65536.0)  # scores arrive x(8*32)^2


def build_nc(n_batch=B_CORE, repeat=1):
    """Build the per-core Bass module. Same program on all cores (SPMD)."""
    nc = bacc.Bacc("TRN2")

    # qT host-prepped as [b, 128, ND*T] bf16: partition p holds d-rows
    # d*128+p of q^T, contiguous per partition (3200B descriptors).
    qx = nc.dram_tensor("qx", [n_batch, 128, XB], U8, kind="ExternalInput")
    wqk = nc.dram_tensor("wqk", [D, 128], FP8, kind="ExternalInput")
    wv8 = nc.dram_tensor("wv8", [D, DK], FP8, kind="ExternalInput")
    wv16 = nc.dram_tensor("wv16", [D, DK], BF16, kind="ExternalInput")
    mask0 = nc.dram_tensor("mask0", [128, T0W], BF16, kind="ExternalInput")
    mask1 = nc.dram_tensor("mask1", [T1W, T1W], BF16, kind="ExternalInput")
    out = nc.dram_tensor("out", [n_batch // 2, 100, 260], BF16, kind="ExternalOutput")

    assert n_batch % 2 == 0
    n_pair = n_batch // 2

    with tile.TileContext(nc) as tc:
        with (
            tc.tile_pool(name="singles", bufs=1) as singles,
            tc.tile_pool(name="qt", bufs=4) as qt_pool,
            tc.tile_pool(name="qk", bufs=3) as qk_pool,
            tc.tile_pool(name="esb", bufs=8) as esb_pool,
            tc.tile_pool(name="vsb", bufs=6) as vsb_pool,
            tc.tile_pool(name="osb", bufs=8) as osb_pool,
            tc.tile_pool(name="ps_qk", bufs=2, space="PSUM") as ps_qk,
            tc.tile_pool(name="ps_v", bufs=2, space="PSUM") as ps_v,
            tc.tile_pool(name="ps_e", bufs=3, space="PSUM") as ps_e,
            tc.tile_pool(name="ps_u", bufs=1, space="PSUM") as ps_u,
        ):
            # ---- constants, loaded once ----
            wqk_sb = singles.tile([128, ND, 128], FP8)
            nc.sync.dma_start(
                out=wqk_sb, in_=wqk.rearrange("(d p) j -> p d j", p=128)
            )
            wv8_sb = singles.tile([128, ND, DK], FP8)
            nc.sync.dma_start(out=wv8_sb, in_=wv8.rearrange("(d p) j -> p d j", p=128))
            wv16_sb = singles.tile([128, ND, DK], BF16)
            nc.sync.dma_start(
                out=wv16_sb, in_=wv16.rearrange("(d p) j -> p d j", p=128)
            )
            m0_sb = singles.tile([128, T0W], BF16)
            nc.sync.dma_start(out=m0_sb, in_=mask0[:, :])
            m1_sb = singles.tile([T1W, T1W], BF16)
            nc.sync.dma_start(out=m1_sb, in_=mask1[:, :])

            def emit_load(b0, nb2):
                """Prefetch batches [b0, b0+nb2) via the Pool SWDGE queue."""
                qt = qt_pool.tile([128, nb2, XB], U8, tag=f"qt{nb2}")
                nc.gpsimd.dma_start(
                    out=qt, in_=qx[b0 : b0 + nb2].rearrange("b p x -> p b x")
                )
                return qt

            def emit_projection(p, qt):
                """Project Q,K and V (fp8 DoubleRow; V rows t<TH redone in
                bf16), evacuate. Returns attention inputs."""
                # fp8 per-partition layout: 4 k-tiles x 400B, each k-tile
                # [j*128+t | 256 + j*72 + (t-128)] so DoubleRow stationaries
                # are contiguous [ [tw,2], [1,tw] ] blocks (ISA requirement).
                q8a = qt.bitcast(FP8).rearrange(
                    "pp b (k y) -> pp k b y", k=ND // 2
                )
                qk_ps = ps_qk.tile([128, 2 * TP], F32)
                qk3 = qk_ps.rearrange("m (b t) -> m b t", b=2)
                for ti, (t0, tw) in enumerate(((0, T0W), (T0W, T1P))):
                    y0 = 0 if ti == 0 else 2 * T0W
                    for k in range(ND // 2):
                        nc.tensor.matmul(
                            qk3[:, :, t0 : t0 + tw],
                            wqk_sb[:, 2 * k : 2 * k + 2, :],
                            q8a[:, k, :, y0 : y0 + 2 * tw].rearrange(
                                "pp b (j t) -> pp j b t", j=2
                            ),
                            start=(k == 0), stop=(k == ND // 2 - 1),
                            perf_mode=mybir.MatmulPerfMode.DoubleRow,
                            skip_group_check=True,
                        )

                # V in [t, dv] rows: fp8 DoubleRow, then bf16 head overwrite
                # (col 64 / 129 are gap columns the ones-memsets fill)
                v_ps = ps_v.tile([128, 2, 130], F32)
                for bi in range(2):
                    for ti, (t0, tw) in enumerate(((0, T0W), (T0W, T1P))):
                        dst = v_ps[0:tw, bi, 65 * ti : 65 * ti + 64]
                        y0 = 0 if ti == 0 else 2 * T0W
                        for k in range(ND // 2):
                            nc.tensor.matmul(
                                dst,
                                q8a[:, k, bi, y0 : y0 + 2 * tw].rearrange(
                                    "pp (j t) -> pp j t", j=2
                                ),
                                wv8_sb[:, 2 * k : 2 * k + 2, :],
                                start=(k == 0), stop=(k == ND // 2 - 1),
                                perf_mode=mybir.MatmulPerfMode.DoubleRow,
                            )
                    for d in range(ND):
                        k, j = d // 2, d % 2
                        h0 = k * KB + 2 * T0W + j * T1P + T1W
                        nc.tensor.matmul(
                            v_ps[0:TH, bi, 0:64],
                            qt[:, bi, h0 : h0 + 2 * TH].bitcast(BF16),
                            wv16_sb[:, d, :],
                            start=(d == 0), stop=(d == ND - 1),
                            skip_group_check=True,
                        )

                # evacuate: Q^T as bf16 moving operand, K^T partition-shifted
                # down to rows 0:64 as the stationary operand
                qk4 = qk_ps.rearrange("m (b t) -> m b t", b=2)
                qm_sb = qk_pool.tile([64, 2, T], BF16, tag="qm")
                nc.scalar.copy(qm_sb, qk4[0:64, :, 0:T])
                kt_sb = qk_pool.tile([64, 2, T], BF16, tag="kt")
                nc.vector.tensor_copy(kt_sb, qk4[64:128, :, 0:T])

                # 256*V + ones columns -> moving operand [v0 | 1 | v1 | 1]
                # (the x256 scale is divided out on the host)
                vaugs = []
                for bi in range(2):
                    va = vsb_pool.tile([128, 130], BF16, tag=f"va{bi}")
                    nc.vector.tensor_copy(va, v_ps[:, bi, :])
                    nc.gpsimd.memset(va[:, 64:65], 1.0)
                    nc.gpsimd.memset(va[0:T1W, 129:130], 1.0)
                    vaugs.append(va)
                return qm_sb, kt_sb, vaugs

            def emit_scores(p, qm_sb, kt_sb):
                """E = exp(K Q^T / 32), causally trimmed + masked, bf16."""
                es = []
                for bi in range(2):
                    e_ps = ps_e.tile([128, T + T1W], F32, tag="e_ps")
                    nc.tensor.matmul(
                        e_ps[:, 0:T],
                        kt_sb[:, bi, 0:T0W],
                        qm_sb[:, bi, 0:T],
                        start=True, stop=True,
                    )
                    nc.tensor.matmul(
                        e_ps[0:T1W, T : T + T1W],
                        kt_sb[:, bi, T0W:T],
                        qm_sb[:, bi, T0W:T],
                        start=True, stop=True,
                    )
                    eb = esb_pool.tile([128, T + T1W], BF16, tag="eb")
                    nc.scalar.activation(
                        eb, e_ps,
                        mybir.ActivationFunctionType.Exp, scale=ESCALE,
                    )
                    e0 = eb[:, 0:T]
                    e1 = eb[0:T1W, T : T + T1W]
                    nc.vector.tensor_mul(e0[:, 0:T0W], e0[:, 0:T0W], m0_sb)
                    nc.gpsimd.tensor_mul(e1, e1, m1_sb)
                    es.append((e0, e1))
                return es

            def emit_out(p, es, vaugs):
                """U = E^T @ [V|1]; store unnormalized bf16 (host divides)."""
                o_sb = osb_pool.tile([100, 260], BF16, tag="o_sb")
                u_ps = ps_u.tile([100, 260], F32, tag="u_ps")
                for bi in range(2):
                    e0, e1 = es[bi]
                    e0p = e0.rearrange("s (t c) -> s t c", c=2)
                    e1p = e1.rearrange("s (t c) -> s t c", c=2)
                    va = vaugs[bi]
                    u2 = u_ps[:, 130 * bi : 130 * bi + 130]
                    for ci in range(2):
                        nc.tensor.matmul(
                            u2[:, 65 * ci : 65 * ci + 65],
                            e0p[:, :, ci],
                            va[:, 0:65],
                            start=True, stop=False,
                            skip_group_check=True,
                        )
                        nc.tensor.matmul(
                            u2[64:100, 65 * ci : 65 * ci + 65],
                            e1p[:, :, ci],
                            va[0:T1W, 65:130],
                            start=False, stop=True,
                            skip_group_check=True,
                        )
                if p % 2 == 0:
                    nc.scalar.copy(o_sb, u_ps)
                else:
                    nc.vector.tensor_copy(o_sb, u_ps)
                nc.sync.dma_start(
                    out=out[p].rearrange("t (b x) -> t b x", b=2), in_=o_sb
                )

            # software pipeline, 4 stages 1 pair apart:
            # load superpair | proj p | scores p-1 | U/out p-2
            for _rep in range(repeat):
                proj_t = {}
                score_t = {}
                q_tiles = {0: emit_load(0, 2)}
                if n_pair > 1:
                    t4 = emit_load(2, 4)
                    q_tiles[1] = t4[:, 0:2, :]
                    q_tiles[2] = t4[:, 2:4, :]
                for p in range(n_pair + 2):
                    if p < n_pair:
                        if p >= 1 and p % 2 == 1 and p + 2 < n_pair:
                            if p + 3 < n_pair:
                                t4 = emit_load(2 * (p + 2), 4)
                                q_tiles[p + 2] = t4[:, 0:2, :]
                                q_tiles[p + 3] = t4[:, 2:4, :]
                            else:
                                q_tiles[p + 2] = emit_load(2 * (p + 2), 2)
                        proj_t[p] = emit_projection(p, q_tiles.pop(p))
                    if p - 2 >= 0:
                        emit_out(p - 2, score_t.pop(p - 2),
                                 proj_t.pop(p - 2)[2])
                    if p - 1 >= 0 and p - 1 < n_pair:
                        qm_sb, kt_sb, _ = proj_t[p - 1]
                        score_t[p - 1] = emit_scores(p - 1, qm_sb, kt_sb)

    nc.compile()
    return nc


def round_f32r(a):
    """Round fp32 to the PE's fp32r format (11-bit mantissa, RNE)."""
    b = np.ascontiguousarray(a, dtype=np.float32).view(np.uint32)
    r = (b + 0x7FF + ((b >> 12) & 1)) & np.uint32(0xFFFFF000)
    return r.astype(np.uint32).view(np.float32)


def _host_inputs(q, Wq, Wk, Wv):
    """Shared (replicated) device inputs + per-core packed qx shards."""
    wqk = np.ascontiguousarray(
        np.concatenate([Wq, Wk], axis=1) * 32.0, dtype=np.float32
    ).astype(ml_dtypes.float8_e4m3fn)
    wv32 = np.ascontiguousarray(np.asarray(Wv) * 32.0, dtype=np.float32)
    wv8 = wv32.astype(ml_dtypes.float8_e4m3fn)
    wv16 = wv32.astype(ml_dtypes.bfloat16)
    t_idx = np.arange(T0W)[None, :]
    m0 = (t_idx >= np.arange(128)[:, None]).astype(ml_dtypes.bfloat16)
    t_idx = (T0W + np.arange(T1W))[None, :]
    m1 = (t_idx >= (T0W + np.arange(T1W))[:, None]).astype(ml_dtypes.bfloat16)
    # qT[b, p, d8, t]: partition p holds rows d8*128+p of q^T
    nb = q.shape[0]
    qT = (
        np.asarray(q, dtype=np.float32)
        .transpose(0, 2, 1)
        .reshape(nb, ND, 128, T)
        .transpose(0, 2, 1, 3)
    )  # [b, 128, ND, T]
    q8 = (qT * 8.0).astype(ml_dtypes.float8_e4m3fn)  # [b, 128, 8, 200]
    qa = q8[..., 0:T0W].reshape(nb, 128, ND // 2, 2 * T0W)
    # tt1 block per (k, j): [72 fp8 | 24B pad = 12 bf16 head values]
    qb = np.zeros((nb, 128, ND, T1P), dtype=np.uint8)
    qb[..., 0:T1W] = q8[..., T0W:T].view(np.uint8)
    q16 = np.ascontiguousarray((qT[:, :, :, 0:TH] * 8.0).astype(ml_dtypes.bfloat16))
    qb[..., T1W:T1P] = q16.view(np.uint8).reshape(nb, 128, ND, 2 * TH)
    qb = qb.reshape(nb, 128, ND // 2, 2 * T1P)
    qx = np.concatenate([qa.view(np.uint8), qb], axis=3).reshape(nb, 128, XB)
    return np.ascontiguousarray(qx), {
        "wqk": wqk, "wv8": wv8, "wv16": wv16, "mask0": m0, "mask1": m1,
    }


_NC_CACHE = {}


def _get_nc(n_batch=B_CORE, repeat=1):
    key = (n_batch, repeat)
    if key not in _NC_CACHE:
        _NC_CACHE[key] = build_nc(n_batch, repeat)
    return _NC_CACHE[key]


def _unpack_out(u):
    """[n_pair, 100, 260] bf16 unnormalized -> [2*n_pair, T, DK] f32."""
    u = u.astype(np.float32).reshape(u.shape[0], 100, 2, 2, 65)
    vals = u[..., 0:64]  # [p, t2, b, ci, 64] = 256 * numerator
    den = u[..., 64:65]
    o = vals / den / 256.0  # [p, t2, b, ci, 64]
    o = o.transpose(0, 2, 1, 3, 4).reshape(u.shape[0] * 2, T, DK)
    return np.ascontiguousarray(o)


def kernel(q, Wq, Wk, Wv):
    q = np.asarray(q, dtype=np.float32)
    qT, shared = _host_inputs(q, np.asarray(Wq), np.asarray(Wk), np.asarray(Wv))

    nc = _get_nc()
    in_maps = [
        {"qx": np.ascontiguousarray(qT[c * B_CORE : (c + 1) * B_CORE]), **shared}
        for c in range(N_CORES)
    ]
    res = run_bass_kernel_spmd(nc, in_maps, core_ids=list(range(N_CORES)))
    return np.concatenate([_unpack_out(r["out"]) for r in res.results], axis=0)
